# revision 22
# baseline (speedup 1.0000x reference)
"""Chamfer-distance criterion kernel for Trainium2 (8 NeuronCores, data-parallel over batch).

Math: the reference's two [B,T,T] pairwise cross-entropy GEMMs collapse exactly
because one side of each GEMM is a (masked) one-hot:

  probs = softmax(logits); p0 = probs[:,:,0]; valid = (t!=0)&(t!=PAD)
  knn_ce(one_hot, xs) = sum_{valid n} clamp(-amax1_n, C1, C0)
     with amax1_n = max_{valid m} (l[m, t_n] - logZ_m)
  knn_ce(xs, one_hot) = sum_n valid_n*(C0*(1-p0) - (C0-C1)*exp(gmax_n - logZ_n))
     with gmax_n = max_{valid j} l[n, t_j], C0 = -log(eps), C1 = -log1p(-(D-1)eps)

So each core needs, per row, only Z = sum(exp(l)) over the full vocab, plus
tiny reductions over host-gathered f32 logit columns at the <=64 target ids
per batch (max commutes with exp; -log(clip(exp(a))) == clamp(-a, C1, C0)).

The Z pass streams the logits as fp8_e4m3 (host cast; ~0.2% rms Z error, far
inside the 2e-2 gate) and splits the vocab between two engines per row-tile:
  cols [0, VA):   ACT exp (double-rate at fp8) + hardware row-accumulate
  cols [VA, V):   DVE Schraudolph exp -- i32(x*2^23/ln2 + B) bitcast to f32
                  IS ~exp(x); one tensor_scalar convert + one accum-sum
"""

import math
import os
import numpy as np
from contextlib import ExitStack

import concourse.bass as bass
import concourse.tile as tile
from concourse import bacc, mybir

# ---- problem constants (hardcoded per contract) ----
B, T, V = 64, 64, 8192
PAD = 8192
EPS = 1e-8
D = V - 1
C0 = float(-math.log(EPS))
C1 = float(-math.log1p(-(D - 1) * EPS))
BIG = 50.0                 # additive log-domain mask; BIG > C0 + max|l| + max logZ

N_CORES = 8
BPC = B // N_CORES          # batches per core = 8
ROWS = BPC * T              # rows per core = 512
P = 128                     # partitions per tile
NT = ROWS // P              # row tiles per core = 4
NJ = 64                     # gather slots: one per target position
VA = int(os.environ.get("KVA", "6016"))   # ACT's column share (mult of 64)
VB = V - VA                               # DVE's column share
SCH_S = float(2.0 ** 23 / math.log(2.0))  # Schraudolph scale
SCH_B = float(127.0 * 2 ** 23 - 486411.0)  # Schraudolph bias (mean-centered)
F32 = mybir.dt.float32
F16 = mybir.dt.float16
F8 = mybir.dt.float8e4
I32 = mybir.dt.int32


def _build_program(reps=1):
    nc = bacc.Bacc("TRN2", target_bir_lowering=False, debug=False)
    x_d = nc.dram_tensor("x", [P, NT * V], F8, kind="ExternalInput").ap()
    g_d = nc.dram_tensor("g", [P, NT, NJ], F32, kind="ExternalInput").ap()
    gm_d = nc.dram_tensor("gm", [P, NT, NJ], F32, kind="ExternalInput").ap()
    hostm_d = nc.dram_tensor("hostm", [P, NT], F32, kind="ExternalInput").ap()
    l0_d = nc.dram_tensor("l0", [P, NT], F32, kind="ExternalInput").ap()
    mval_d = nc.dram_tensor("mval", [P, NT], F32, kind="ExternalInput").ap()
    epw_d = nc.dram_tensor("epw", [P, NT], F32, kind="ExternalInput").ap()
    ehw_d = nc.dram_tensor("ehw", [P, NT], F32, kind="ExternalInput").ap()
    mvt_d = nc.dram_tensor("mvt", [P, NT], F32, kind="ExternalInput").ap()
    wcat_d = nc.dram_tensor("wcat", [P, 2 * NT], F32, kind="ExternalInput").ap()
    ident_d = nc.dram_tensor("ident", [P, 128], F32, kind="ExternalInput").ap()
    ones_d = nc.dram_tensor("ones", [P, 1], F32, kind="ExternalInput").ap()
    out_d = nc.dram_tensor("out", [1, 2], F32, kind="ExternalOutput").ap()

    AF = mybir.ActivationFunctionType
    OP = mybir.AluOpType

    with tile.TileContext(nc) as tc:
        with ExitStack() as ctx:
            const = ctx.enter_context(tc.tile_pool(name="const", bufs=1))
            xp = ctx.enter_context(tc.tile_pool(name="xp", bufs=2))
            epool = ctx.enter_context(tc.tile_pool(name="ep", bufs=2))
            ipool = ctx.enter_context(tc.tile_pool(name="ip", bufs=2))
            spool = ctx.enter_context(tc.tile_pool(name="sp", bufs=2))
            apool = ctx.enter_context(tc.tile_pool(name="apool", bufs=2))
            small = ctx.enter_context(tc.tile_pool(name="small", bufs=2))
            tpp = ctx.enter_context(tc.tile_pool(name="tpp", bufs=6, space="PSUM"))
            finp = ctx.enter_context(tc.tile_pool(name="finp", bufs=2, space="PSUM"))

            # constants / marshaled inputs (DMA'd once; reused every rep)
            ident = const.tile([P, 128], F32)
            nc.sync.dma_start(ident[:], ident_d[:])
            ones = const.tile([P, 1], F32)
            nc.sync.dma_start(ones[:], ones_d[:])
            g3 = const.tile([P, NT, NJ], F32)
            nc.sync.dma_start(g3[:], g_d[:])
            gm3 = const.tile([P, NT, NJ], F32)
            nc.sync.dma_start(gm3[:], gm_d[:])
            hostm = const.tile([P, NT], F32)
            nc.sync.dma_start(hostm[:], hostm_d[:])
            l0 = const.tile([P, NT], F32)
            nc.sync.dma_start(l0[:], l0_d[:])
            mval = const.tile([P, NT], F32)
            nc.sync.dma_start(mval[:], mval_d[:])
            epw = const.tile([P, NT], F32)
            nc.sync.dma_start(epw[:], epw_d[:])
            ehw = const.tile([P, NT], F32)
            nc.sync.dma_start(ehw[:], ehw_d[:])
            mvt = const.tile([P, NT], F32)
            nc.sync.dma_start(mvt[:], mvt_d[:])
            wcat = const.tile([P, 2 * NT], F32)
            nc.sync.dma_start(wcat[:], wcat_d[:])
            # gmax over valid targets: pure function of const inputs
            gmx = const.tile([P, NT], F32)
            nc.vector.tensor_reduce(gmx[:], gm3[:],
                                    axis=mybir.AxisListType.X, op=OP.max)

            def trace_stream():
                """Issue one rep's Z-pass; return tiles the epilogue needs."""
                xt4 = xp.tile([P, NT, V], F8, tag="xt")
                nc.sync.dma_start(xt4[:], x_d[:])
                zA = small.tile([P, NT], F32, tag="zA")
                zB = small.tile([P, NT], F32, tag="zB")
                for i in range(NT):
                    eta = epool.tile([P, VA], F16, tag="eta")
                    nc.scalar.activation(eta[:], xt4[:, i, 0:VA], AF.Exp,
                                         accum_out=zA[:, i:i + 1])
                    it32 = ipool.tile([P, VB], I32, tag="it")
                    nc.vector.tensor_scalar(
                        out=it32[:], in0=xt4[:, i, VA:V], scalar1=SCH_S,
                        scalar2=SCH_B, op0=OP.mult, op1=OP.add)
                    st = spool.tile([P, VB], F32, tag="st")
                    nc.vector.tensor_scalar(
                        out=st[:], in0=it32[:].bitcast(F32), scalar1=0.0,
                        scalar2=None, op0=OP.add, op1=OP.add,
                        accum_out=zB[:, i:i + 1])
                return zA, zB

            def trace_epilogue(sv):
                zA, zB = sv
                zcol = small.tile([P, NT], F32, tag="zcol")
                nc.vector.tensor_add(zcol[:], zA[:], zB[:])
                lz = small.tile([P, NT], F32, tag="lz")
                nc.scalar.activation(lz[:], zcol[:], AF.Ln)
                # lzm = -logZ - BIG*invalid_row   (hostm = 0 / -BIG)
                lzm = small.tile([P, NT], F32, tag="lzm")
                nc.vector.scalar_tensor_tensor(
                    out=lzm[:], in0=lz[:], scalar=-1.0, in1=hostm[:],
                    op0=OP.mult, op1=OP.add)
                ecat = small.tile([P, 2 * NT], F32, tag="ecat")
                nc.vector.tensor_add(ecat[:, 0:NT], gmx[:], lzm[:])
                # logp0 = l0 - logZ (unmasked)
                nc.vector.scalar_tensor_tensor(
                    out=ecat[:, NT:2 * NT], in0=lz[:], scalar=-1.0, in1=l0[:],
                    op0=OP.mult, op1=OP.add)
                pcat = small.tile([P, 2 * NT], F32, tag="pcat")
                nc.scalar.activation(pcat[:], ecat[:], AF.Exp)  # [m2 | p0]
                onem = small.tile([P, NT], F32, tag="onem")
                nc.vector.tensor_scalar(
                    out=onem[:], in0=pcat[:, NT:2 * NT], scalar1=-1.0,
                    scalar2=1.0, op0=OP.mult, op1=OP.add)       # 1 - p0
                # log1m overwrites ecat[:, 0:NT] (e1 already consumed by Exp)
                # -> ecat becomes [log1m | logp0], multiplied by wcat=[ehw|epw]
                nc.scalar.activation(ecat[:, 0:NT], onem[:], AF.Ln)
                tmp2 = small.tile([P, NT], F32, tag="tmp2")
                # tmp2 = (1-p0) - ((C0-C1)/C0)*m2  (C0 folded into mvalC0)
                nc.vector.scalar_tensor_tensor(
                    out=tmp2[:], in0=pcat[:, 0:NT], scalar=-(C0 - C1) / C0,
                    in1=onem[:], op0=OP.mult, op1=OP.add)
                rcat = small.tile([P, 3 * NT], F32, tag="rcat")
                nc.vector.tensor_mul(rcat[:, 0:NT], tmp2[:], mval[:])
                nc.vector.tensor_mul(rcat[:, NT:3 * NT], ecat[:], wcat[:])

                # ---- term1: cross-row max of a = g - logZ_row - BIG*invalid
                # at4 pair c holds [a_{2c} | a_{2c+1}]; one [128,128] transpose
                # per pair, one segmented [128,2,64] reduce -> m1cat[:, 2c:2c+2]
                at4 = apool.tile([P, NT * NJ], F32, tag="at4")
                for i in range(NT):
                    nc.vector.tensor_scalar_add(at4[:, NJ * i:NJ * (i + 1)],
                                                g3[:, i, :],
                                                scalar1=lzm[:, i:i + 1])
                m1cat = small.tile([P, NT], F32, tag="m1cat")
                for c in range(2):
                    tp = tpp.tile([P, 2, 64], F32, tag="tp")
                    nc.tensor.transpose(tp[:], at4[:, 128 * c:128 * c + 128],
                                        ident[:])
                    nc.vector.tensor_reduce(m1cat[:, 2 * c:2 * c + 2], tp[:],
                                            axis=mybir.AxisListType.X,
                                            op=OP.max)
                # t1 = clamp(-amax1, C1, C0); mask by valid_n (mvt layout)
                t1a = small.tile([P, NT], F32, tag="t1a")
                nc.vector.tensor_scalar(
                    out=t1a[:], in0=m1cat[:], scalar1=-1.0, scalar2=C1,
                    op0=OP.mult, op1=OP.max)
                t1b = small.tile([P, NT], F32, tag="t1b")
                nc.vector.tensor_scalar_min(t1b[:], t1a[:], C0)
                t1col = small.tile([P, NT], F32, tag="t1col")
                nc.vector.tensor_mul(t1col[:], t1b[:], mvt[:])

                # ---- final partition-dim sums via matmul with ones ----
                psA = finp.tile([1, 3 * NT], F32, tag="psA")
                nc.tensor.matmul(out=psA[:], lhsT=ones[:], rhs=rcat[:],
                                 start=True, stop=False)
                nc.tensor.matmul(out=psA[0:1, 0:NT], lhsT=ones[:],
                                 rhs=t1col[:], start=False, stop=True)
                out_t = small.tile([1, 2], F32, tag="out_t")
                nc.vector.tensor_reduce(out_t[:, 0:1], psA[0:1, 0:NT],
                                        axis=mybir.AxisListType.X, op=OP.add)
                nc.vector.tensor_reduce(out_t[:, 1:2], psA[0:1, NT:3 * NT],
                                        axis=mybir.AxisListType.X, op=OP.add)
                nc.sync.dma_start(out_d[:], out_t[:])

            prev = None
            for rep in range(reps):
                cur = trace_stream()
                if prev is not None:
                    trace_epilogue(prev)
                prev = cur
            trace_epilogue(prev)

    nc.compile()
    return nc


def _prep_core_inputs(logits, targets, core):
    """Host-side marshaling for one core (batches core*BPC .. core*BPC+BPC-1)."""
    import ml_dtypes
    b0 = core * BPC
    lg = np.asarray(logits[b0:b0 + BPC], dtype=np.float32)  # [BPC, T, V]
    # [P, NT*V] fp8: x[p, i*V + v] = logit of row i*128+p, col v
    x = np.ascontiguousarray(
        lg.reshape(NT, P, V).transpose(1, 0, 2).reshape(P, NT * V)
    ).astype(ml_dtypes.float8_e4m3)
    tg = np.asarray(targets[b0:b0 + BPC])
    valid = (tg != 0) & (tg != PAD)                         # [BPC, T]
    tgc = np.where(valid, tg, 0).astype(np.int64)
    validf = valid.astype(np.float32)
    ep = (tg == 0).astype(np.float32)
    ep_w = -0.5 / (B * (ep.sum(axis=1) + EPS))              # [BPC]
    eh_w = -0.5 / (B * (validf.sum(axis=1) + EPS))

    g = np.zeros((P, NT, NJ), dtype=np.float32)
    gm = np.zeros((P, NT, NJ), dtype=np.float32)
    hostm = np.zeros((P, NT), dtype=np.float32)
    l0 = np.zeros((P, NT), dtype=np.float32)
    mval = np.zeros((P, NT), dtype=np.float32)
    epw = np.zeros((P, NT), dtype=np.float32)
    ehw = np.zeros((P, NT), dtype=np.float32)
    p = np.arange(P)
    r = p % 64
    for i in range(NT):
        bl = 2 * i + p // 64                                # [P]
        g[:, i, :] = lg[bl[:, None], r[:, None], tgc[bl, :]]
        gm[:, i, :] = g[:, i, :] + (validf[bl, :] - 1.0) * BIG
        hostm[:, i] = (validf[bl, r] - 1.0) * BIG
        l0[:, i] = lg[bl, r, 0]
        mval[:, i] = validf[bl, r] * C0
        epw[:, i] = ep[bl, r] * ep_w[bl]
        ehw[:, i] = validf[bl, r] * eh_w[bl]
    mvt = np.zeros((P, NT), dtype=np.float32)
    q = np.arange(P)
    for c in range(2):
        for h in range(2):
            mvt[:, 2 * c + h] = validf[4 * c + 2 * (q // 64) + h, q % 64]
    wcat = np.concatenate([ehw, epw], axis=1)               # [P, 2*NT]
    ident = np.zeros((P, P), dtype=np.float32)
    ident[np.arange(P), np.arange(P)] = 1.0
    ones = np.ones((P, 1), dtype=np.float32)
    return {"x": x, "g": g, "gm": gm, "hostm": hostm, "l0": l0, "mval": mval,
            "epw": epw, "ehw": ehw, "mvt": mvt, "wcat": wcat, "ident": ident,
            "ones": ones}


_CACHE = {}


def _get_runner():
    """Build the Bass program and a cached 8-core PJRT executable."""
    if "runner" in _CACHE:
        return _CACHE["runner"]
    import jax
    from jax.sharding import Mesh, PartitionSpec
    from jax.experimental.shard_map import shard_map
    from concourse import bass2jax

    nc = _build_program()
    bass2jax.install_neuronx_cc_hook()

    part_name = nc.partition_id_tensor.name if nc.partition_id_tensor else None
    in_names, out_names, out_avals, zero_outs = [], [], [], []
    for alloc in nc.m.functions[0].allocations:
        if not isinstance(alloc, mybir.MemoryLocationSet):
            continue
        name = alloc.memorylocations[0].name
        if alloc.kind == "ExternalInput":
            if name != part_name:
                in_names.append(name)
        elif alloc.kind == "ExternalOutput":
            out_names.append(name)
            shape = tuple(alloc.tensor_shape)
            dtype = mybir.dt.np(alloc.dtype)
            out_avals.append(jax.core.ShapedArray(shape, dtype))
            zero_outs.append(np.zeros(shape, dtype))
    n_params = len(in_names)
    all_names = in_names + out_names
    if part_name is not None:
        all_names = all_names + [part_name]

    def _body(*args):
        operands = list(args)
        if part_name is not None:
            operands.append(bass2jax.partition_id_tensor())
        outs = bass2jax._bass_exec_p.bind(
            *operands,
            out_avals=tuple(out_avals),
            in_names=tuple(all_names),
            out_names=tuple(out_names),
            lowering_input_output_aliases=(),
            sim_require_finite=True,
            sim_require_nnan=True,
            nc=nc,
        )
        return tuple(outs)

    devices = jax.devices()[:N_CORES]
    mesh = Mesh(np.asarray(devices), ("core",))
    donate = tuple(range(n_params, n_params + len(out_names)))
    sharded = jax.jit(
        shard_map(_body, mesh=mesh,
                  in_specs=(PartitionSpec("core"),) * (n_params + len(out_names)),
                  out_specs=(PartitionSpec("core"),) * len(out_names),
                  check_rep=False),
        donate_argnums=donate, keep_unused=True)

    runner = (sharded, in_names, out_names, zero_outs)
    _CACHE["runner"] = runner
    return runner


def run_device(in_maps):
    """Run the SPMD program; in_maps is a list of N_CORES dicts."""
    sharded, in_names, out_names, zero_outs = _get_runner()
    concat_in = [
        np.concatenate([in_maps[c][n] for c in range(N_CORES)], axis=0)
        for n in in_names
    ]
    concat_zero = [
        np.zeros((N_CORES * z.shape[0], *z.shape[1:]), z.dtype) for z in zero_outs
    ]
    out_arrs = sharded(*concat_in, *concat_zero)
    out0 = np.asarray(out_arrs[0]).reshape(N_CORES, 1, 2)
    return out0


def kernel(logits, targets):
    logits = np.asarray(logits)
    targets = np.asarray(targets)
    in_maps = [_prep_core_inputs(logits, targets, c) for c in range(N_CORES)]
    outs = run_device(in_maps)                             # [N_CORES, 1, 2]
    label = outs[:, 0, 0].sum(dtype=np.float64)
    eos = outs[:, 0, 1].sum(dtype=np.float64)
    return (np.float32(label), np.float32(eos))
